# revision 1
# baseline (speedup 1.0000x reference)
"""CEMA kernel for Trainium2: batch-mean + EMA scan over sequence.

Computes, for x[B=8, S=4096, D=2048] fp32:
    m = mean(x, axis=0)                       # [S, D]
    ema_t = a*ema_{t-1} + (1-a)*m_t  (scan)   # [S, D]
    out = broadcast(ema, [B, S, D])

Distribution: the EMA scan is elementwise in D, so D is sharded across the
8 cores (DC=256 columns each) — no collectives needed. Each core receives
its x[:, :, d_lo:d_hi] slab rearranged host-side to [S, B*DC] so the DMA
reads 8KB contiguous per partition.

Per-core algorithm: blocks of L=127 sequence steps. The batch sum is a
3-level in-place halving tree on DVE. The scan block is ONE PE matmul:
    rhs [128, 256] = [carry_row ; M_rows]  (carry is contraction row 0)
    lhsT[k, i] holds alpha-power coefficients, with column 0 duplicating
    the last step so the next block's carry lands on PSUM partition 0.
The carry handoff is then a same-partition ACT copy (PSUM row 0 -> next
rhs tile row 0) — no cross-partition moves anywhere.
"""

import sys

for _p in ("/opt/trn_rl_repo", "/root/.axon_site/_ro/trn_rl_repo"):
    if _p not in sys.path:
        sys.path.append(_p)

import numpy as np

import concourse.bass as bass  # noqa: F401  (AP helpers)
import concourse.tile as tile
from concourse import bacc, mybir
from concourse import bass_utils

ALPHA = 0.99
B, S, D = 8, 4096, 2048
NCORES = 8
DC = D // NCORES          # 256 columns per core
L = 127                   # full block length (carry occupies row 0 of 128)
LT = S - (S // L) * L     # tail block length (32)
F32 = mybir.dt.float32


def _make_lhsT(lb: int) -> np.ndarray:
    """Stationary matrix for one scan block of lb steps.

    out[i, d] = sum_k lhsT[k, i] * rhs[k, d], rhs row 0 = carry (ema before
    block), rows 1..lb = batch SUMS of x (the /B and (1-a) are folded here).
    out rows 1..lb = ema at steps t0..t0+lb-1; out row 0 duplicates the last
    step so the next carry can be read from PSUM partition 0.
    """
    t = np.zeros((lb + 1, 128), dtype=np.float64)
    scale = (1.0 - ALPHA) / B
    for i in range(1, lb + 1):
        t[0, i] = ALPHA ** i
        for k in range(1, i + 1):
            t[k, i] = ALPHA ** (i - k) * scale
    t[:, 0] = t[:, lb]
    return np.ascontiguousarray(t.astype(np.float32))


def build_nc():
    nc = bacc.Bacc(
        "TRN2", target_bir_lowering=False, debug=False, enable_asserts=False
    )
    xs = nc.dram_tensor("xs", [S, B * DC], F32, kind="ExternalInput").ap()
    tm = nc.dram_tensor("tmain", [L + 1, 128], F32, kind="ExternalInput").ap()
    tt = nc.dram_tensor("ttail", [LT + 1, 128], F32, kind="ExternalInput").ap()
    ys = nc.dram_tensor("ys", [S, DC], F32, kind="ExternalOutput").ap()

    with tile.TileContext(nc) as tc:
        with (
            tc.tile_pool(name="const", bufs=1) as const_pool,
            tc.tile_pool(name="xs", bufs=6) as xs_pool,
            tc.tile_pool(name="psum", bufs=6, space="PSUM") as psum_pool,
            tc.tile_pool(name="y", bufs=4) as y_pool,
        ):
            tm_sb = const_pool.tile([L + 1, 128], F32)
            nc.sync.dma_start(tm_sb[:, :], tm)
            tt_sb = const_pool.tile([LT + 1, 128], F32)
            nc.sync.dma_start(tt_sb[:, :], tt)
            # Zeros for the carry row's upper free region: engine ops must
            # start at partition 0/32/64/96, so the halving tree includes the
            # carry row; zeros there make the tree a no-op on it.
            z_sb = const_pool.tile([1, (B - 1) * DC], F32)
            nc.vector.memset(z_sb[:, :], 0.0)

            prev_ps = None
            t0 = 0
            while t0 < S:
                lb = min(L, S - t0)
                k = lb + 1
                xt = xs_pool.tile([128, B * DC], F32)
                nc.sync.dma_start(xt[1 : 1 + lb, :], xs[t0 : t0 + lb, :])
                nc.sync.dma_start(xt[0:1, DC : B * DC], z_sb[0:1, :])
                if prev_ps is None:
                    nc.vector.memset(xt[0:1, 0:DC], 0.0)
                else:
                    nc.scalar.copy(xt[0:1, 0:DC], prev_ps[0:1, 0:DC])
                # batch sum: in-place halving tree over the b-major free axis
                # (rows 0..lb; row 0 is carry + zeros, unchanged by the adds)
                w = B * DC
                while w > DC:
                    h = w // 2
                    nc.vector.tensor_add(
                        xt[0:k, 0:h], xt[0:k, 0:h], xt[0:k, h:w]
                    )
                    w = h
                ps = psum_pool.tile([128, DC], F32)
                lhsT = tm_sb if lb == L else tt_sb
                nc.tensor.matmul(
                    ps[:, :], lhsT[0:k, :], xt[0:k, 0:DC], start=True, stop=True
                )
                yt = y_pool.tile([128, DC], F32)
                nc.vector.tensor_copy(yt[:, :], ps[:, :])
                nc.sync.dma_start(ys[t0 : t0 + lb, :], yt[1 : 1 + lb, :])
                prev_ps = ps
                t0 += lb
    nc.compile()
    return nc


_NC_CACHE = None


def _get_nc():
    global _NC_CACHE
    if _NC_CACHE is None:
        _NC_CACHE = build_nc()
    return _NC_CACHE


def make_in_maps(x: np.ndarray) -> list[dict]:
    x = np.asarray(x, dtype=np.float32)
    tm_np = _make_lhsT(L)
    tt_np = _make_lhsT(LT)
    in_maps = []
    for i in range(NCORES):
        xsl = x[:, :, i * DC : (i + 1) * DC]  # [B, S, DC]
        xs_i = np.ascontiguousarray(xsl.transpose(1, 0, 2)).reshape(S, B * DC)
        in_maps.append({"xs": xs_i, "tmain": tm_np, "ttail": tt_np})
    return in_maps


def run(x: np.ndarray, trace: bool = False, **kw):
    """Returns (out [B,S,D] fp32, BassKernelResults)."""
    nc = _get_nc()
    res = bass_utils.run_bass_kernel_spmd(
        nc, make_in_maps(x), core_ids=list(range(NCORES)), trace=trace, **kw
    )
    emas = np.concatenate([r["ys"] for r in res.results], axis=1)  # [S, D]
    out = np.broadcast_to(emas[None, :, :], (B, S, D))
    return out, res


def kernel(x: np.ndarray) -> np.ndarray:
    out, _ = run(x, trace=False)
    return out



# revision 3
# speedup vs baseline: 3.4009x; 3.4009x over previous
"""CEMA kernel for Trainium2: batch-mean + EMA scan over sequence.

Computes, for x[B=8, S=4096, D=2048] fp32:
    m = mean(x, axis=0)                       # [S, D]
    ema_t = a*ema_{t-1} + (1-a)*m_t  (scan)   # [S, D]
    out = broadcast(ema, [B, S, D])

Distribution: the EMA scan is elementwise in D, so D is sharded across the
8 cores (DC=256 columns each) — no collectives needed. Each core receives
its x[:, :, d_lo:d_hi] slab rearranged host-side to [S, B*DC] so the DMA
reads 8KB contiguous per partition.

Per-core algorithm: blocks of L=127 sequence steps. The batch sum is a
3-level in-place halving tree on DVE. The scan block is ONE PE matmul:
    rhs [128, 256] = [carry_row ; M_rows]  (carry is contraction row 0)
    lhsT[k, i] holds alpha-power coefficients, with column 0 duplicating
    the last step so the next block's carry lands on PSUM partition 0.
The carry handoff is then a same-partition ACT copy (PSUM row 0 -> next
rhs tile row 0) — no cross-partition moves anywhere.
"""

import sys

for _p in ("/opt/trn_rl_repo", "/root/.axon_site/_ro/trn_rl_repo"):
    if _p not in sys.path:
        sys.path.append(_p)

import numpy as np

import concourse.bass as bass  # noqa: F401  (AP helpers)
import concourse.tile as tile
from concourse import bacc, mybir
from concourse import bass_utils

ALPHA = 0.99
B, S, D = 8, 4096, 2048
NCORES = 8
DC = D // NCORES          # 256 columns per core
L = 127                   # full block length (carry occupies row 0 of 128)
LT = S - (S // L) * L     # tail block length (32)
F32 = mybir.dt.float32


def _make_lhsT(lb: int) -> np.ndarray:
    """Stationary matrix for one scan block of lb steps.

    out[i, d] = sum_k lhsT[k, i] * rhs[k, d], rhs row 0 = carry (ema before
    block), rows 1..lb = batch SUMS of x (the /B and (1-a) are folded here).
    out rows 1..lb = ema at steps t0..t0+lb-1; out row 0 duplicates the last
    step so the next carry can be read from PSUM partition 0.
    """
    t = np.zeros((lb + 1, 128), dtype=np.float64)
    scale = (1.0 - ALPHA) / B
    for i in range(1, lb + 1):
        t[0, i] = ALPHA ** i
        for k in range(1, i + 1):
            t[k, i] = ALPHA ** (i - k) * scale
    t[:, 0] = t[:, lb]
    return np.ascontiguousarray(t.astype(np.float32))


def build_nc():
    nc = bacc.Bacc(
        "TRN2", target_bir_lowering=False, debug=False, enable_asserts=False
    )
    xs = nc.dram_tensor("xs", [S, B * DC], F32, kind="ExternalInput").ap()
    tm = nc.dram_tensor("tmain", [L + 1, 128], F32, kind="ExternalInput").ap()
    tt = nc.dram_tensor("ttail", [LT + 1, 128], F32, kind="ExternalInput").ap()
    ys = nc.dram_tensor("ys", [S, DC], F32, kind="ExternalOutput").ap()

    with tile.TileContext(nc) as tc:
        with (
            tc.tile_pool(name="const", bufs=1) as const_pool,
            tc.tile_pool(name="xs", bufs=6) as xs_pool,
            tc.tile_pool(name="psum", bufs=6, space="PSUM") as psum_pool,
            tc.tile_pool(name="y", bufs=4) as y_pool,
        ):
            tm_sb = const_pool.tile([L + 1, 128], F32)
            nc.sync.dma_start(tm_sb[:, :], tm)
            tt_sb = const_pool.tile([LT + 1, 128], F32)
            nc.sync.dma_start(tt_sb[:, :], tt)
            # Zeros for the carry row's upper free region: engine ops must
            # start at partition 0/32/64/96, so the halving tree includes the
            # carry row; zeros there make the tree a no-op on it.
            z_sb = const_pool.tile([1, (B - 1) * DC], F32)
            nc.vector.memset(z_sb[:, :], 0.0)

            prev_ps = None
            t0 = 0
            while t0 < S:
                lb = min(L, S - t0)
                k = lb + 1
                xt = xs_pool.tile([128, B * DC], F32)
                # SWDGE (gpsimd): one op is swizzled across all 16 SDMA
                # engines; HWDGE (sync) put every descriptor on ONE engine
                # (~26 GB/s measured) for this pattern.
                nc.gpsimd.dma_start(xt[1 : 1 + lb, :], xs[t0 : t0 + lb, :])
                nc.sync.dma_start(xt[0:1, DC : B * DC], z_sb[0:1, :])
                if prev_ps is None:
                    nc.vector.memset(xt[0:1, 0:DC], 0.0)
                else:
                    nc.scalar.copy(xt[0:1, 0:DC], prev_ps[0:1, 0:DC])
                # batch sum: in-place halving tree over the b-major free axis
                # (rows 0..lb; row 0 is carry + zeros, unchanged by the adds)
                w = B * DC
                while w > DC:
                    h = w // 2
                    nc.vector.tensor_add(
                        xt[0:k, 0:h], xt[0:k, 0:h], xt[0:k, h:w]
                    )
                    w = h
                ps = psum_pool.tile([128, DC], F32)
                lhsT = tm_sb if lb == L else tt_sb
                nc.tensor.matmul(
                    ps[:, :], lhsT[0:k, :], xt[0:k, 0:DC], start=True, stop=True
                )
                yt = y_pool.tile([128, DC], F32)
                nc.vector.tensor_copy(yt[:, :], ps[:, :])
                nc.gpsimd.dma_start(ys[t0 : t0 + lb, :], yt[1 : 1 + lb, :])
                prev_ps = ps
                t0 += lb
    nc.compile()
    return nc


_NC_CACHE = None


def _get_nc():
    global _NC_CACHE
    if _NC_CACHE is None:
        _NC_CACHE = build_nc()
    return _NC_CACHE


def make_in_maps(x: np.ndarray) -> list[dict]:
    x = np.asarray(x, dtype=np.float32)
    tm_np = _make_lhsT(L)
    tt_np = _make_lhsT(LT)
    in_maps = []
    for i in range(NCORES):
        xsl = x[:, :, i * DC : (i + 1) * DC]  # [B, S, DC]
        xs_i = np.ascontiguousarray(xsl.transpose(1, 0, 2)).reshape(S, B * DC)
        in_maps.append({"xs": xs_i, "tmain": tm_np, "ttail": tt_np})
    return in_maps


def run(x: np.ndarray, trace: bool = False, **kw):
    """Returns (out [B,S,D] fp32, BassKernelResults)."""
    nc = _get_nc()
    res = bass_utils.run_bass_kernel_spmd(
        nc, make_in_maps(x), core_ids=list(range(NCORES)), trace=trace, **kw
    )
    emas = np.concatenate([r["ys"] for r in res.results], axis=1)  # [S, D]
    out = np.broadcast_to(emas[None, :, :], (B, S, D))
    return out, res


def kernel(x: np.ndarray) -> np.ndarray:
    out, _ = run(x, trace=False)
    return out

